# revision 8
# baseline (speedup 1.0000x reference)
"""Trainium2 Bass kernel for the MetaLayer GNN message-passing problem.

Strategy (8 NeuronCores, SPMD, two launches):
  Launch 1 (edges, sharded E/8 per core): 2-layer edge MLP over
    concat([edge_attr, x[row], x[col], u_g, face[f0], face[f1]]).
    All six 128-wide input blocks are staged host-side in feature-major
    layout (gathers done on host via np.take); the per-graph u block is
    folded into a per-graph layer-1 bias.  Also emits per-graph edge
    aggregates for the global update.
  Host glue: assemble full e' from the per-core shards, CSR-sort the four
    scatter streams (row, col, face0, face1) and pre-gather the padded
    streams so launch 2 reads purely contiguous data.
  Launch 2 (nodes F/8 + faces N/8 + globals, sharded per core): segmented
    sums computed with one-hot selection matrices (DVE is_equal vs iota)
    contracted on the PE into PSUM; then the node / face / global MLPs.
    Graphs are block-aligned so every aggregate stays core-local - no
    cross-core communication at all.

kernel(**inputs) takes the FULL inputs and returns the FULL outputs
(x', edge_attr', u', face') exactly like reference.reference().
"""

import os
import sys
import types

for _p in ("/opt/trn_rl_repo", "/root/.axon_site"):
    if _p not in sys.path and os.path.isdir(_p):
        sys.path.insert(0, _p)

import numpy as np

import concourse.bass as bass
import concourse.mybir as mybir
from concourse.tile import TileContext
from concourse.vector_clock import ScopedClock
from concourse.bass_utils import run_bass_kernel_spmd

F32 = mybir.dt.float32
AF = mybir.ActivationFunctionType
ALU = mybir.AluOpType

NCORES = 8


class _TC(TileContext):
    pass


_MAX_WAITS = int(os.environ.get("BASS_GNN_MAXWAITS", "1"))


def _split_sync_waits(nc, max_waits=None):
    """This walrus build rejects instructions carrying more than a couple of
    sync waits.  Hoist excess semaphore waits onto nop instructions inserted
    immediately before, on the same engine (equivalent semantics: the engine
    stream is sequential)."""
    if max_waits is None:
        max_waits = _MAX_WAITS
    uid = [0]
    for f in nc.m.functions:
        for bb in f.blocks:
            insts = bb.instructions
            i = 0
            while i < len(insts):
                inst = insts[i]
                si = inst.sync_info
                if si is not None and si.on_wait and len(si.on_wait) > max_waits:
                    waits = list(si.on_wait)
                    sem_w = [w for w in waits if w.sync_type == "semaphore"]
                    other = [w for w in waits if w.sync_type != "semaphore"]
                    n_keep = max(0, max_waits - len(other))
                    keep = other + sem_w[:n_keep]
                    move = sem_w[n_keep:]
                    if not move or len(keep) > max_waits:
                        i += 1
                        continue
                    inst.sync_info = mybir.SyncInfo(
                        on_wait=keep, on_update=list(si.on_update)
                    )
                    for w in move:
                        uid[0] += 1
                        nop = mybir.InstNoOp(
                            name=f"waitsplit_{uid[0]}_{inst.name}",
                            engine=inst.engine,
                            sync_info=mybir.SyncInfo(on_wait=[w], on_update=[]),
                            bass_nofuse=True,
                        )
                        try:
                            nc.register_instruction(nop, overwrite=True)
                        except Exception:
                            pass
                        insts.insert(i, nop)
                        i += 1
                i += 1
    return nc


# --------------------------------------------------------------------------
# Launch 1: edge MLP
# --------------------------------------------------------------------------

def _build_edge_nc(Ec, Gc, ET=512):
    """Per-core edge-update kernel.  Ec edges per core, Gc graphs per core,
    ET edges per tile (one graph never straddles a tile)."""
    n_tiles = Ec // ET
    tiles_per_graph = n_tiles // Gc

    nc = bass.Bass()
    srcs = [
        nc.declare_dram_parameter(nm, [128, Ec], F32, isOutput=False)
        for nm in ("ea_t", "xr_t", "xc_t", "f0_t", "f1_t")
    ]
    w1 = nc.declare_dram_parameter("ew1", [128, 5 * 128], F32, isOutput=False)
    b1g = nc.declare_dram_parameter("eb1g", [128, Gc], F32, isOutput=False)
    w2 = nc.declare_dram_parameter("ew2", [128, 128], F32, isOutput=False)
    b2 = nc.declare_dram_parameter("eb2", [128, 1], F32, isOutput=False)
    eo = nc.declare_dram_parameter("eo_t", [128, Ec], F32, isOutput=True)
    eagg = nc.declare_dram_parameter("eagg", [128, Gc], F32, isOutput=True)

    with _TC(nc) as tc:
        with (
            tc.tile_pool(name="wp", bufs=1) as wp,
            tc.tile_pool(name="sp", bufs=3) as sp,
            tc.tile_pool(name="acc", bufs=1) as accp,
            tc.tile_pool(name="pp", bufs=2, space="PSUM") as pp,
        ):
            w1_sb = wp.tile([128, 5 * 128], F32)
            nc.sync.dma_start(out=w1_sb[:], in_=w1[:])
            w2_sb = wp.tile([128, 128], F32)
            nc.sync.dma_start(out=w2_sb[:], in_=w2[:])
            b1g_sb = wp.tile([128, Gc], F32)
            nc.sync.dma_start(out=b1g_sb[:], in_=b1g[:])
            b2_sb = wp.tile([128, 1], F32)
            nc.sync.dma_start(out=b2_sb[:], in_=b2[:])

            eagg_acc = accp.tile([128, Gc], F32)
            nc.vector.memset(eagg_acc[:], 0.0)

            for t in range(n_tiles):
                g = t // tiles_per_graph
                sl = slice(t * ET, (t + 1) * ET)
                ins = []
                for j, src in enumerate(srcs):
                    tl = sp.tile([128, ET], F32, tag=f"in{j}")
                    nc.sync.dma_start(out=tl[:], in_=src[:, sl])
                    ins.append(tl)
                h1p = pp.tile([128, ET], F32, tag="h1p")
                for j, tl in enumerate(ins):
                    nc.tensor.matmul(
                        out=h1p[:],
                        lhsT=w1_sb[:, j * 128:(j + 1) * 128],
                        rhs=tl[:],
                        start=(j == 0),
                        stop=(j == 4),
                    )
                h1s = sp.tile([128, ET], F32, tag="h1s")
                nc.scalar.activation(h1s[:], h1p[:], AF.Relu, bias=b1g_sb[:, g:g + 1])
                h2p = pp.tile([128, ET], F32, tag="h2p")
                nc.tensor.matmul(out=h2p[:], lhsT=w2_sb[:], rhs=h1s[:],
                                 start=True, stop=True)
                h2s = sp.tile([128, ET], F32, tag="h2s")
                part = sp.tile([128, 1], F32, tag="part")
                nc.scalar.activation(h2s[:], h2p[:], AF.Identity,
                                     bias=b2_sb[:, 0:1], accum_out=part[:])
                nc.sync.dma_start(out=eo[:, sl], in_=h2s[:])
                nc.vector.tensor_tensor(
                    out=eagg_acc[:, g:g + 1], in0=eagg_acc[:, g:g + 1],
                    in1=part[:], op=ALU.add,
                )
            nc.sync.dma_start(out=eagg[:], in_=eagg_acc[:])
    return _split_sync_waits(nc)


# --------------------------------------------------------------------------
# Launch 2: node update + face update + global update
# --------------------------------------------------------------------------

def _build_node_face_nc(Nc, Fc, Gc, C, Cf):
    """Per-core kernel for phases 2-4.  Nc nodes / Fc faces / Gc graphs per
    core; C (Cf) 128-row chunks per 128-node (128-face) tile."""
    n_nt = Nc // 128
    n_ft = Fc // 128
    nt_per_graph = n_nt // Gc
    ft_per_graph = n_ft // Gc

    nc = bass.Bass()
    P = lambda nm, shp: nc.declare_dram_parameter(nm, shp, F32, isOutput=False)
    O = lambda nm, shp: nc.declare_dram_parameter(nm, shp, F32, isOutput=True)

    sstream = P("sstream", [n_nt * C * 128, 128])
    rstream = P("rstream", [n_nt * C * 128, 128])
    slocid = P("slocid_t", [128, n_nt * C])
    rlocid = P("rlocid_t", [128, n_nt * C])
    fsstream = P("fsstream", [n_ft * Cf * 128, 128])
    frstream = P("frstream", [n_ft * Cf * 128, 128])
    fslocid = P("fslocid_t", [128, n_ft * Cf])
    frlocid = P("frlocid_t", [128, n_ft * Cf])
    x_t = P("x_t", [128, Nc])
    face_t = P("face_t", [128, Fc])
    maskf = P("maskf", [128, Fc])
    iota = P("iota", [128, 128])
    nw1 = P("nw1", [128, 3 * 128])
    nb1g = P("nb1g", [128, Gc])
    nw2 = P("nw2", [128, 128])
    nb2 = P("nb2", [128, 1])
    fw1 = P("fw1", [128, 3 * 128])
    fb1g = P("fb1g", [128, Gc])
    fw2 = P("fw2", [128, 128])
    fb2 = P("fb2", [128, 1])
    gw1 = P("gw1", [128, 4 * 128])
    gb1 = P("gb1", [128, 1])
    gw2 = P("gw2", [128, 128])
    gb2 = P("gb2", [128, 1])
    u16 = P("u16", [128, Gc])
    eagg = P("eagg", [128, Gc])

    xo = O("xo_t", [128, Nc])
    fo = O("fo_t", [128, Fc])
    uo = O("uo_t", [128, Gc])

    with _TC(nc) as tc:
        with (
            tc.tile_pool(name="wp", bufs=1) as wp,
            tc.tile_pool(name="sp", bufs=4) as sp,
            tc.tile_pool(name="acc", bufs=1) as accp,
            tc.tile_pool(name="ppseg", bufs=4, space="PSUM") as pp_seg,
            tc.tile_pool(name="ppmlp", bufs=2, space="PSUM") as pp_mlp,
        ):
            def _load_w(src, shape, tag):
                t = wp.tile(shape, F32, tag=tag)
                nc.sync.dma_start(out=t[:], in_=src[:])
                return t

            iota_sb = _load_w(iota, [128, 128], "w_iota")
            slocid_sb = _load_w(slocid, [128, n_nt * C], "w_slocid")
            rlocid_sb = _load_w(rlocid, [128, n_nt * C], "w_rlocid")
            fslocid_sb = _load_w(fslocid, [128, n_ft * Cf], "w_fslocid")
            frlocid_sb = _load_w(frlocid, [128, n_ft * Cf], "w_frlocid")
            nw1_sb = _load_w(nw1, [128, 3 * 128], "w_nw1")
            nb1g_sb = _load_w(nb1g, [128, Gc], "w_nb1g")
            nw2_sb = _load_w(nw2, [128, 128], "w_nw2")
            nb2_sb = _load_w(nb2, [128, 1], "w_nb2")
            fw1_sb = _load_w(fw1, [128, 3 * 128], "w_fw1")
            fb1g_sb = _load_w(fb1g, [128, Gc], "w_fb1g")
            fw2_sb = _load_w(fw2, [128, 128], "w_fw2")
            fb2_sb = _load_w(fb2, [128, 1], "w_fb2")
            gw1_sb = _load_w(gw1, [128, 4 * 128], "w_gw1")
            gb1_sb = _load_w(gb1, [128, 1], "w_gb1")
            gw2_sb = _load_w(gw2, [128, 128], "w_gw2")
            gb2_sb = _load_w(gb2, [128, 1], "w_gb2")
            u16_sb = _load_w(u16, [128, Gc], "w_u16")
            eagg_sb = _load_w(eagg, [128, Gc], "w_eagg")
            maskf_sb = _load_w(maskf, [128, Fc], "w_maskf")

            nagg_acc = accp.tile([128, Gc], F32)
            nc.vector.memset(nagg_acc[:], 0.0)
            fagg_acc = accp.tile([128, Gc], F32)
            nc.vector.memset(fagg_acc[:], 0.0)

            def _segsum(stream, locid_sb, tile_idx, nchunks, psum_tag):
                """One-hot matmul segmented sum for one 128-target tile."""
                ps = pp_seg.tile([128, 128], F32, tag="seg")
                for c in range(nchunks):
                    ch = tile_idx * nchunks + c
                    st = sp.tile([128, 128], F32, tag=psum_tag + "_st")
                    nc.sync.dma_start(
                        out=st[:], in_=stream[ch * 128:(ch + 1) * 128, :]
                    )
                    oh = sp.tile([128, 128], F32, tag=psum_tag + "_oh")
                    nc.vector.tensor_tensor(
                        out=oh[:],
                        in0=locid_sb[:, ch:ch + 1].to_broadcast([128, 128]),
                        in1=iota_sb[:],
                        op=ALU.is_equal,
                    )
                    nc.tensor.matmul(out=ps[:], lhsT=st[:], rhs=oh[:],
                                     start=(c == 0), stop=(c == nchunks - 1))
                out_sb = sp.tile([128, 128], F32, tag=psum_tag + "_sb")
                nc.vector.tensor_copy(out=out_sb[:], in_=ps[:])
                return out_sb

            # ---- node update ----
            for t in range(n_nt):
                g = t // nt_per_graph
                sl = slice(t * 128, (t + 1) * 128)
                sent_sb = _segsum(sstream, slocid_sb, t, C, "sent")
                recv_sb = _segsum(rstream, rlocid_sb, t, C, "recv")
                xt_sb = sp.tile([128, 128], F32, tag="xt")
                nc.sync.dma_start(out=xt_sb[:], in_=x_t[:, sl])
                h1p = pp_mlp.tile([128, 128], F32, tag="h1p")
                nc.tensor.matmul(out=h1p[:], lhsT=nw1_sb[:, 0:128], rhs=xt_sb[:],
                                 start=True, stop=False)
                nc.tensor.matmul(out=h1p[:], lhsT=nw1_sb[:, 128:256], rhs=sent_sb[:],
                                 start=False, stop=False)
                nc.tensor.matmul(out=h1p[:], lhsT=nw1_sb[:, 256:384], rhs=recv_sb[:],
                                 start=False, stop=True)
                h1s = sp.tile([128, 128], F32, tag="nh1s")
                nc.scalar.activation(h1s[:], h1p[:], AF.Relu,
                                     bias=nb1g_sb[:, g:g + 1])
                h2p = pp_mlp.tile([128, 128], F32, tag="h2p")
                nc.tensor.matmul(out=h2p[:], lhsT=nw2_sb[:], rhs=h1s[:],
                                 start=True, stop=True)
                xo_sb = sp.tile([128, 128], F32, tag="xo")
                part = sp.tile([128, 1], F32, tag="npart")
                nc.scalar.activation(xo_sb[:], h2p[:], AF.Identity,
                                     bias=nb2_sb[:, 0:1], accum_out=part[:])
                nc.sync.dma_start(out=xo[:, sl], in_=xo_sb[:])
                nc.vector.tensor_tensor(
                    out=nagg_acc[:, g:g + 1], in0=nagg_acc[:, g:g + 1],
                    in1=part[:], op=ALU.add,
                )

            # ---- face update ----
            for t in range(n_ft):
                g = t // ft_per_graph
                sl = slice(t * 128, (t + 1) * 128)
                fs_sb = _segsum(fsstream, fslocid_sb, t, Cf, "fs")
                fr_sb = _segsum(frstream, frlocid_sb, t, Cf, "fr")
                ft_sb = sp.tile([128, 128], F32, tag="ft")
                nc.sync.dma_start(out=ft_sb[:], in_=face_t[:, sl])
                h1p = pp_mlp.tile([128, 128], F32, tag="h1p")
                nc.tensor.matmul(out=h1p[:], lhsT=fw1_sb[:, 0:128], rhs=ft_sb[:],
                                 start=True, stop=False)
                nc.tensor.matmul(out=h1p[:], lhsT=fw1_sb[:, 128:256], rhs=fs_sb[:],
                                 start=False, stop=False)
                nc.tensor.matmul(out=h1p[:], lhsT=fw1_sb[:, 256:384], rhs=fr_sb[:],
                                 start=False, stop=True)
                h1s = sp.tile([128, 128], F32, tag="fh1s")
                nc.scalar.activation(h1s[:], h1p[:], AF.Relu,
                                     bias=fb1g_sb[:, g:g + 1])
                h2p = pp_mlp.tile([128, 128], F32, tag="h2p")
                nc.tensor.matmul(out=h2p[:], lhsT=fw2_sb[:], rhs=h1s[:],
                                 start=True, stop=True)
                fpre_sb = sp.tile([128, 128], F32, tag="fpre")
                nc.scalar.activation(fpre_sb[:], h2p[:], AF.Identity,
                                     bias=fb2_sb[:, 0:1])
                fo_sb = sp.tile([128, 128], F32, tag="fo")
                nc.vector.tensor_tensor(out=fo_sb[:], in0=fpre_sb[:],
                                        in1=maskf_sb[:, sl], op=ALU.mult)
                fo2_sb = sp.tile([128, 128], F32, tag="fo2")
                part = sp.tile([128, 1], F32, tag="fpart")
                nc.scalar.activation(fo2_sb[:], fo_sb[:], AF.Identity,
                                     bias=0.0, accum_out=part[:])
                nc.sync.dma_start(out=fo[:, sl], in_=fo2_sb[:])
                nc.vector.tensor_tensor(
                    out=fagg_acc[:, g:g + 1], in0=fagg_acc[:, g:g + 1],
                    in1=part[:], op=ALU.add,
                )

            # ---- global update ----
            gh1p = pp_mlp.tile([128, Gc], F32, tag="h1p")
            nc.tensor.matmul(out=gh1p[:], lhsT=gw1_sb[:, 0:128], rhs=u16_sb[:],
                             start=True, stop=False)
            nc.tensor.matmul(out=gh1p[:], lhsT=gw1_sb[:, 128:256], rhs=nagg_acc[:],
                             start=False, stop=False)
            nc.tensor.matmul(out=gh1p[:], lhsT=gw1_sb[:, 256:384], rhs=eagg_sb[:],
                             start=False, stop=False)
            nc.tensor.matmul(out=gh1p[:], lhsT=gw1_sb[:, 384:512], rhs=fagg_acc[:],
                             start=False, stop=True)
            gh1s = sp.tile([128, Gc], F32, tag="gh1s")
            nc.scalar.activation(gh1s[:], gh1p[:], AF.Relu, bias=gb1_sb[:, 0:1])
            gh2p = pp_mlp.tile([128, Gc], F32, tag="h2p")
            nc.tensor.matmul(out=gh2p[:], lhsT=gw2_sb[:], rhs=gh1s[:],
                             start=True, stop=True)
            uo_sb = sp.tile([128, Gc], F32, tag="uo")
            nc.scalar.activation(uo_sb[:], gh2p[:], AF.Identity,
                                 bias=gb2_sb[:, 0:1])
            nc.sync.dma_start(out=uo[:], in_=uo_sb[:])
    return _split_sync_waits(nc)


# --------------------------------------------------------------------------
# Host-side stream construction
# --------------------------------------------------------------------------

def _build_streams(tgt, T, e_pad, C=None):
    """CSR-sort edges by target and pre-gather padded 128-row chunks.

    tgt: [E] int targets in [0, T).  e_pad: [E+1, 128] with zero sentinel row.
    Returns (stream [n_tiles, C*128, 128], locid [n_tiles, C*128], C).
    """
    Etot = tgt.shape[0]
    order = np.argsort(tgt, kind="stable")
    stgt = tgt[order]
    n_tiles = T // 128
    counts = np.bincount(tgt // 128, minlength=n_tiles)
    if C is None:
        C = max(1, int(-(-counts.max() // 128)))
    slots = C * 128
    eids = np.full((n_tiles, slots), Etot, dtype=np.int64)
    locid = np.full((n_tiles, slots), 255.0, dtype=np.float32)
    starts = np.concatenate([[0], np.cumsum(counts)])
    for t in range(n_tiles):
        ccnt = int(counts[t])
        s = int(starts[t])
        eids[t, :ccnt] = order[s:s + ccnt]
        locid[t, :ccnt] = (stgt[s:s + ccnt] - t * 128).astype(np.float32)
    stream = e_pad[eids.reshape(-1)].reshape(n_tiles, slots, 128)
    return stream, locid, C


def _locid_t(locid_core, C):
    """[tiles, C*128] -> [128, tiles*C] with column = tile*C + chunk."""
    tpc = locid_core.shape[0]
    return np.ascontiguousarray(
        locid_core.reshape(tpc, C, 128).transpose(2, 0, 1).reshape(128, tpc * C)
    )


def _chunk_counts_ok(tgt, T, cap=None):
    return True


# --------------------------------------------------------------------------
# Pure-numpy fallback (used only if the batch structure is not the
# equal-block layout this kernel is specialized for)
# --------------------------------------------------------------------------

def _np_mlp(h, W1, b1, W2, b2):
    return np.maximum(h @ W1 + b1, 0.0) @ W2 + b2


def _np_segsum(data, seg, n):
    out = np.zeros((n, data.shape[1]), dtype=data.dtype)
    np.add.at(out, seg, data)
    return out


def _numpy_ref(x, edge_attr, u, face, edge_index, face_index, node_batch,
               edge_batch, face_batch, num_nodes, num_edges, num_faces,
               face_mask, eW1, eb1, eW2, eb2, nW1, nb1, nW2, nb2,
               fW1, fb1, fW2, fb2, gW1, gb1, gW2, gb2):
    n_node, n_edge, n_face = x.shape[0], edge_attr.shape[0], face.shape[0]
    n_graph = u.shape[0]
    row, col = edge_index[0], edge_index[1]
    ge = np.repeat(np.arange(n_graph), num_edges)
    e_in = np.concatenate(
        [edge_attr, x[row], x[col], u[ge],
         face[face_index[0]], face[face_index[1]]], axis=1)
    edge_attr = _np_mlp(e_in, eW1, eb1, eW2, eb2)
    sent = _np_segsum(edge_attr, row, n_node)
    recv = _np_segsum(edge_attr, col, n_node)
    gn = np.repeat(np.arange(n_graph), num_nodes)
    x = _np_mlp(np.concatenate([x, sent, recv, u[gn]], axis=1),
                nW1, nb1, nW2, nb2)
    f_sent = _np_segsum(edge_attr, face_index[0], n_face)
    f_recv = _np_segsum(edge_attr, face_index[1], n_face)
    gf = np.repeat(np.arange(n_graph), num_faces)
    face = _np_mlp(np.concatenate([face, f_sent, f_recv, u[gf]], axis=1),
                   fW1, fb1, fW2, fb2)
    face = np.where(face_mask[:, None], np.zeros((), face.dtype), face)
    node_aggr = _np_segsum(x, node_batch, n_graph)
    edge_aggr = _np_segsum(edge_attr, edge_batch, n_graph)
    face_aggr = _np_segsum(face, face_batch, n_graph)
    u = _np_mlp(np.concatenate([u, node_aggr, edge_aggr, face_aggr], axis=1),
                gW1, gb1, gW2, gb2)
    return x, edge_attr, u, face


# --------------------------------------------------------------------------
# Main entry
# --------------------------------------------------------------------------

_NC_CACHE = {}
_TRACE = bool(os.environ.get("BASS_GNN_TRACE"))
LAST_TIMES = {}


def _maybe_install_ntff_hook():
    if "antenv.axon_hooks" in sys.modules:
        return
    try:
        import antenv  # noqa: F401
        mod = types.ModuleType("antenv.axon_hooks")
        _hook = [None]
        mod.set_axon_ntff_profile_hook = lambda h: _hook.__setitem__(0, h)
        mod.get_axon_ntff_profile_hook = lambda: _hook[0]
        sys.modules["antenv.axon_hooks"] = mod
        from trn_agent_boot.trn_boot import _ntff_profile_via_ctypes
        mod.set_axon_ntff_profile_hook(
            _ntff_profile_via_ctypes("/opt/axon/libaxon_pjrt.so")
        )
    except Exception:
        pass


def _run(nc, in_maps, label):
    kwargs = {}
    if _TRACE:
        _maybe_install_ntff_hook()
        kwargs["trace"] = True
    res = run_bass_kernel_spmd(nc, in_maps, list(range(NCORES)), **kwargs)
    if _TRACE:
        LAST_TIMES[label] = res.exec_time_ns
    return res.results


def _equal_blocks(inputs, N, E, F, B):
    try:
        nn_ = np.asarray(inputs["num_nodes"])
        ne_ = np.asarray(inputs["num_edges"])
        nf_ = np.asarray(inputs["num_faces"])
        nb_ = np.asarray(inputs["node_batch"])
        eb_ = np.asarray(inputs["edge_batch"])
        fb_ = np.asarray(inputs["face_batch"])
        if N % B or E % B or F % B:
            return False
        if not (np.all(nn_ == N // B) and np.all(ne_ == E // B)
                and np.all(nf_ == F // B)):
            return False
        if not np.array_equal(nb_, np.repeat(np.arange(B, dtype=nb_.dtype), N // B)):
            return False
        if not np.array_equal(eb_, np.repeat(np.arange(B, dtype=eb_.dtype), E // B)):
            return False
        if not np.array_equal(fb_, np.repeat(np.arange(B, dtype=fb_.dtype), F // B)):
            return False
        # tiling divisibility requirements
        if N % (NCORES * 128) or E % (NCORES * 512) or F % (NCORES * 128):
            return False
        if B % NCORES:
            return False
        if (E // B) % 512 or (N // B) % 128 or (F // B) % 128:
            return False
        return True
    except Exception:
        return False


def kernel(**inputs):
    x = np.ascontiguousarray(np.asarray(inputs["x"], np.float32))
    edge_attr = np.ascontiguousarray(np.asarray(inputs["edge_attr"], np.float32))
    u = np.ascontiguousarray(np.asarray(inputs["u"], np.float32))
    face = np.ascontiguousarray(np.asarray(inputs["face"], np.float32))
    edge_index = np.asarray(inputs["edge_index"])
    face_index = np.asarray(inputs["face_index"])
    face_mask = np.asarray(inputs["face_mask"])
    W = {k: np.ascontiguousarray(np.asarray(inputs[k], np.float32))
         for k in ("eW1", "eb1", "eW2", "eb2", "nW1", "nb1", "nW2", "nb2",
                   "fW1", "fb1", "fW2", "fb2", "gW1", "gb1", "gW2", "gb2")}

    N, D = x.shape
    E = edge_attr.shape[0]
    F = face.shape[0]
    B = u.shape[0]

    if D != 128 or not _equal_blocks(inputs, N, E, F, B):
        return _numpy_ref(
            x, edge_attr, u, face, edge_index, face_index,
            np.asarray(inputs["node_batch"]), np.asarray(inputs["edge_batch"]),
            np.asarray(inputs["face_batch"]), np.asarray(inputs["num_nodes"]),
            np.asarray(inputs["num_edges"]), np.asarray(inputs["num_faces"]),
            face_mask, W["eW1"], W["eb1"], W["eW2"], W["eb2"],
            W["nW1"], W["nb1"], W["nW2"], W["nb2"],
            W["fW1"], W["fb1"], W["fW2"], W["fb2"],
            W["gW1"], W["gb1"], W["gW2"], W["gb2"])

    Ec, Nc, Fc, Gc = E // NCORES, N // NCORES, F // NCORES, B // NCORES
    row = edge_index[0].astype(np.int64)
    col = edge_index[1].astype(np.int64)
    fi0 = face_index[0].astype(np.int64)
    fi1 = face_index[1].astype(np.int64)

    # ---- launch 1: edge MLP ----
    key1 = ("edge", Ec, Gc)
    if key1 not in _NC_CACHE:
        _NC_CACHE[key1] = _build_edge_nc(Ec, Gc)
    nc1 = _NC_CACHE[key1]

    # per-graph layer-1 bias: eb1 + u @ eW1[384:512]  (u is e_in block 3)
    eb1g = (W["eb1"][None, :] + u @ W["eW1"][384:512]).astype(np.float32)  # [B,128]
    ew1_host = np.ascontiguousarray(
        np.concatenate([W["eW1"][0:128], W["eW1"][128:256], W["eW1"][256:384],
                        W["eW1"][512:640], W["eW1"][640:768]], axis=1)
    )  # [128, 5*128]: blocks ea, xr, xc, f0, f1 (u folded into bias)

    in_maps1 = []
    for c in range(NCORES):
        sl = slice(c * Ec, (c + 1) * Ec)
        in_maps1.append({
            "ea_t": np.ascontiguousarray(edge_attr[sl].T),
            "xr_t": np.ascontiguousarray(x[row[sl]].T),
            "xc_t": np.ascontiguousarray(x[col[sl]].T),
            "f0_t": np.ascontiguousarray(face[fi0[sl]].T),
            "f1_t": np.ascontiguousarray(face[fi1[sl]].T),
            "ew1": ew1_host,
            "eb1g": np.ascontiguousarray(eb1g[c * Gc:(c + 1) * Gc].T),
            "ew2": W["eW2"],
            "eb2": np.ascontiguousarray(W["eb2"][:, None]),
            "partition_id": np.array([c], np.int32),
        })
    res1 = _run(nc1, in_maps1, "launch1")
    e_new = np.concatenate([r["eo_t"] for r in res1], axis=1).T  # [E, 128]
    e_new = np.ascontiguousarray(e_new)

    # ---- host glue: CSR streams ----
    e_pad = np.vstack([e_new, np.zeros((1, 128), np.float32)])
    s_stream, s_locid, C = _build_streams(row, N, e_pad)
    r_stream, r_locid, C2 = _build_streams(col, N, e_pad)
    C = max(C, C2)
    if C2 < C:
        r_stream, r_locid, _ = _build_streams(col, N, e_pad, C)
    if s_stream.shape[1] < C * 128:
        s_stream, s_locid, _ = _build_streams(row, N, e_pad, C)
    fs_stream, fs_locid, Cf = _build_streams(fi0, F, e_pad)
    fr_stream, fr_locid, Cf2 = _build_streams(fi1, F, e_pad)
    Cf = max(Cf, Cf2)
    if Cf2 < Cf:
        fr_stream, fr_locid, _ = _build_streams(fi1, F, e_pad, Cf)
    if fs_stream.shape[1] < Cf * 128:
        fs_stream, fs_locid, _ = _build_streams(fi0, F, e_pad, Cf)

    key2 = ("nf", Nc, Fc, Gc, C, Cf)
    if key2 not in _NC_CACHE:
        _NC_CACHE[key2] = _build_node_face_nc(Nc, Fc, Gc, C, Cf)
    nc2 = _NC_CACHE[key2]

    nb1g = (W["nb1"][None, :] + u @ W["nW1"][384:512]).astype(np.float32)
    fb1g = (W["fb1"][None, :] + u @ W["fW1"][384:512]).astype(np.float32)
    nw1_host = np.ascontiguousarray(W["nW1"][0:384].reshape(3, 128, 128)
                                    .transpose(1, 0, 2).reshape(128, 384))
    fw1_host = np.ascontiguousarray(W["fW1"][0:384].reshape(3, 128, 128)
                                    .transpose(1, 0, 2).reshape(128, 384))
    gw1_host = np.ascontiguousarray(W["gW1"].reshape(4, 128, 128)
                                    .transpose(1, 0, 2).reshape(128, 512))
    iota_host = np.ascontiguousarray(
        np.broadcast_to(np.arange(128, dtype=np.float32), (128, 128)))
    maskf_full = (~face_mask.astype(bool)).astype(np.float32)

    n_nt_c = Nc // 128
    n_ft_c = Fc // 128
    in_maps2 = []
    for c in range(NCORES):
        nsl = slice(c * n_nt_c, (c + 1) * n_nt_c)
        fsl = slice(c * n_ft_c, (c + 1) * n_ft_c)
        gsl = slice(c * Gc, (c + 1) * Gc)
        in_maps2.append({
            "sstream": np.ascontiguousarray(
                s_stream[nsl].reshape(-1, 128)),
            "rstream": np.ascontiguousarray(
                r_stream[nsl].reshape(-1, 128)),
            "slocid_t": _locid_t(s_locid[nsl], C),
            "rlocid_t": _locid_t(r_locid[nsl], C),
            "fsstream": np.ascontiguousarray(
                fs_stream[fsl].reshape(-1, 128)),
            "frstream": np.ascontiguousarray(
                fr_stream[fsl].reshape(-1, 128)),
            "fslocid_t": _locid_t(fs_locid[fsl], Cf),
            "frlocid_t": _locid_t(fr_locid[fsl], Cf),
            "x_t": np.ascontiguousarray(x[c * Nc:(c + 1) * Nc].T),
            "face_t": np.ascontiguousarray(face[c * Fc:(c + 1) * Fc].T),
            "maskf": np.ascontiguousarray(np.broadcast_to(
                maskf_full[c * Fc:(c + 1) * Fc], (128, Fc))),
            "iota": iota_host,
            "nw1": nw1_host,
            "nb1g": np.ascontiguousarray(nb1g[gsl].T),
            "nw2": W["nW2"],
            "nb2": np.ascontiguousarray(W["nb2"][:, None]),
            "fw1": fw1_host,
            "fb1g": np.ascontiguousarray(fb1g[gsl].T),
            "fw2": W["fW2"],
            "fb2": np.ascontiguousarray(W["fb2"][:, None]),
            "gw1": gw1_host,
            "gb1": np.ascontiguousarray(W["gb1"][:, None]),
            "gw2": W["gW2"],
            "gb2": np.ascontiguousarray(W["gb2"][:, None]),
            "u16": np.ascontiguousarray(u[gsl].T),
            "eagg": res1[c]["eagg"],
            "partition_id": np.array([c], np.int32),
        })
    res2 = _run(nc2, in_maps2, "launch2")

    x_new = np.ascontiguousarray(
        np.concatenate([r["xo_t"] for r in res2], axis=1).T)
    f_new = np.ascontiguousarray(
        np.concatenate([r["fo_t"] for r in res2], axis=1).T)
    u_new = np.ascontiguousarray(
        np.concatenate([r["uo_t"] for r in res2], axis=1).T)
    return x_new, e_new, u_new, f_new
